# revision 3
# baseline (speedup 1.0000x reference)
"""CrossAttentionHead TRN2 kernel.

Full inputs -> full output. Shards batch (B=8) across 8 NeuronCores,
one batch element per core (pure data parallel, no collectives).

Layout choice: each core's x shard is staged host-side as xT = x.T
([E, S]) in bfloat16, so the kernel streams it straight into the
e-on-partitions layout every matmul needs -- no on-chip transpose
pass over x, and half the HBM traffic of fp32.

Per-core algorithm (xT: [E=768, S=2048] bf16, W*: [E, H=128] bf16):
  qT   = Wq.T @ xT + bq                    ([H, S], weights stationary)
  kT   = Wk.T @ xT + bk
  vT   = Wv.T @ xT + bv  -> vN = transpose(vT)   ([S, H] natural)
  for each sq block (512 wide):
    for each sk tile pair (2x128):
      sT   = kT_tile.T @ qT_block          (scores TRANSPOSED [sk, sq])
      es   = exp(sT / sqrt(E))             (ScalarE, scale fused, bf16 out)
      acc += es                            (DVE bf16, for row sums)
      oT  += vN_tile.T @ es                (PV accumulate, [H, sq])
    rowsum = ones.T @ acc                  ([1, sq] via PE, ones stationary)
    rsT    = transpose(rowsum)             (PE, [sq,1] tiles)
    out    = transpose(oT) * (1/rsT)       -> DMA

All matmul operands are bfloat16 (PE streams 1 cyc/row vs fp32r's
measured ~2 cyc/row, and LDWEIGHTS drops 4x vs fp32). Accumulation
stays fp32 in PSUM. Softmax skips max-subtraction: energy/sqrt(768)
~ N(0, 0.41^2) so exp is safely in range.
"""

import sys

if '/opt/trn_rl_repo' not in sys.path:
    sys.path.insert(0, '/opt/trn_rl_repo')

import numpy as np

B, S, E, H = 8, 2048, 768, 128
NCORES = 8
ST = S // 128          # 16 sequence tiles
EC = E // 128          # 6 embed chunks
QB = 4                 # sq blocks
QW = S // QB           # 512 sq block width
SCALE = float(1.0 / np.sqrt(np.float32(E)))

_CACHE = {}


def _build():
    import concourse.bacc as bacc
    import concourse.mybir as mybir
    import concourse.tile as tile
    from concourse.masks import make_identity

    dt = mybir.dt
    f32 = dt.float32
    bf16 = dt.bfloat16
    AF = mybir.ActivationFunctionType

    nc = bacc.Bacc(None, target_bir_lowering=False)
    xT_d = nc.dram_tensor("xT", [E, S], bf16, kind="ExternalInput")
    w_d = {}
    b_d = {}
    for nm in ("q", "k", "v"):
        w_d[nm] = nc.dram_tensor(f"W{nm}", [E, H], bf16, kind="ExternalInput")
        b_d[nm] = nc.dram_tensor(f"b{nm}", [H], f32, kind="ExternalInput")
    out_d = nc.dram_tensor("out", [S, H], f32, kind="ExternalOutput")

    with tile.TileContext(nc) as tc:
        with tc.tile_pool(name="const", bufs=1) as constp, \
             tc.tile_pool(name="big", bufs=1) as bigp:
            identb = constp.tile([128, 128], bf16)
            make_identity(nc, identb[:])
            ident1 = constp.tile([1, 1], f32)
            nc.vector.memset(ident1[:], 1.0)
            ones = constp.tile([128, 1], bf16)
            nc.vector.memset(ones[:], 1.0)

            # HAM warm-up: junk matmuls flip the PE clock gate to 8/8
            # (~3.4us of activity) while the input DMAs stream in.
            junk = constp.tile([128, 512], bf16, name="warm_junk")
            nc.vector.memset(junk[:], 0.0)
            with tc.tile_pool(name="warm_ps", bufs=1, space="PSUM") as wmp:
                wps = wmp.tile([128, 512], f32, tag="warm")
                for _ in range(8):
                    nc.tensor.matmul(wps[:], identb[:], junk[:],
                                     start=True, stop=True)
                wsb = constp.tile([128, 512], f32, name="warm_sink")
                nc.vector.tensor_copy(wsb[:], wps[:])

            w_mm = {}
            b_sb = {}
            for nm in ("q", "k", "v"):
                w_mm[nm] = constp.tile([128, EC, H], bf16, name=f"w_{nm}")
                nc.sync.dma_start(
                    out=w_mm[nm][:],
                    in_=w_d[nm].rearrange("(c p) d -> p c d", p=128))

            xT = []
            for c in range(EC):
                t = bigp.tile([128, S], bf16, name=f"xT{c}")
                for n in range(2):
                    nc.sync.dma_start(
                        out=t[:, n * 1024:(n + 1) * 1024],
                        in_=xT_d[c * 128:(c + 1) * 128,
                                 n * 1024:(n + 1) * 1024])
                xT.append(t)

            for nm in ("q", "k", "v"):
                b_sb[nm] = constp.tile([128, 1], f32, name=f"b_{nm}")
                nc.sync.dma_start(out=b_sb[nm][:], in_=b_d[nm][:, None])

            # Projections, split per 512-wide n block: qT/kT/vT = W.T@xT + b
            qT = [bigp.tile([128, QW], bf16, name=f"qT{n}") for n in range(4)]
            kT = [bigp.tile([128, QW], bf16, name=f"kT{n}") for n in range(4)]
            vT = [bigp.tile([128, QW], bf16, name=f"vT{n}") for n in range(4)]
            # q/k first with chunk-outer accumulation: every psum tile
            # advances as each xT chunk's DMA lands (no stall on chunk 5)
            with tc.tile_pool(name="proj_ps", bufs=1, space="PSUM") as projp:
                ps_qk = {(nm, n): projp.tile([128, QW], f32,
                                             name=f"ps_{nm}{n}", tag=f"p{nm}{n}")
                         for nm in ("q", "k") for n in range(4)}
                for c in range(EC):
                    for nm in ("q", "k"):
                        for n in range(4):
                            nc.tensor.matmul(
                                ps_qk[(nm, n)][:], w_mm[nm][:, c, :],
                                xT[c][:, n * 512:(n + 1) * 512],
                                start=(c == 0), stop=(c == EC - 1))
                for nm, dst in (("q", qT), ("k", kT)):
                    for n in range(4):
                        nc.vector.tensor_scalar_add(
                            dst[n][:], ps_qk[(nm, n)][:], b_sb[nm][:])
                for n in range(4):
                    ps = projp.tile([128, QW], f32, name=f"ps_v{n}",
                                    tag=f"pq{n}")
                    for c in range(EC):
                        nc.tensor.matmul(
                            ps[:], w_mm["v"][:, c, :],
                            xT[c][:, n * 512:(n + 1) * 512],
                            start=(c == 0), stop=(c == EC - 1))
                    nc.scalar.activation(
                        vT[n][:], ps[:], AF.Identity,
                        bias=b_sb["v"][:], scale=1.0)

            # v natural [S, H], one tile per sk tile
            vN = [bigp.tile([128, H], bf16, name=f"vN{t}") for t in range(ST)]
            with tc.tile_pool(name="vt_ps", bufs=4, space="PSUM") as vtp:
                for t in range(ST):
                    pt = vtp.tile([128, 128], bf16, tag="vt")
                    nc.tensor.transpose(
                        pt[:], vT[t // 4][:, (t % 4) * 128:(t % 4 + 1) * 128],
                        identb[:])
                    nc.vector.tensor_copy(vN[t][:], pt[:])

            # Main attention loop; kt pairs share one 1024-wide psum tile
            # so exp runs at 1024 elems/op
            with tc.tile_pool(name="s_ps", bufs=2, space="PSUM") as sp, \
                 tc.tile_pool(name="o_ps", bufs=2, space="PSUM") as op, \
                 tc.tile_pool(name="f_ps", bufs=2, space="PSUM") as fp, \
                 tc.tile_pool(name="es_sb", bufs=4) as esp, \
                 tc.tile_pool(name="acc_sb", bufs=3) as accp, \
                 tc.tile_pool(name="o_sb", bufs=3) as osp, \
                 tc.tile_pool(name="small", bufs=4) as smp, \
                 tc.tile_pool(name="fin", bufs=4) as finp:
                for qb in range(QB):
                    oT_ps = op.tile([128, QW], f32, tag="opv")
                    acc2 = accp.tile([128, 2 * QW], bf16, tag="acc")
                    # Software pipeline: PV for pair kp-1 is emitted AFTER
                    # the scores for pair kp, so the exp-dependent PV wait
                    # never blocks the next scores pair in PE program order.
                    es_prev = None
                    for kp in range(ST // 2):
                        kt0, kt1 = 2 * kp, 2 * kp + 1
                        s_ps = sp.tile([128, 2 * QW], f32, tag="s")
                        for i, kt in ((0, kt0), (1, kt1)):
                            nc.tensor.matmul(
                                s_ps[:, i * QW:(i + 1) * QW],
                                kT[kt // 4][:, (kt % 4) * 128:(kt % 4 + 1) * 128],
                                qT[qb][:], start=True, stop=True)
                        if es_prev is not None:
                            epv, kpp = es_prev
                            for i, kt in ((0, 2 * kpp), (1, 2 * kpp + 1)):
                                nc.tensor.matmul(
                                    oT_ps[:], vN[kt][:],
                                    epv[:, i * QW:(i + 1) * QW],
                                    start=(kt == 0), stop=False)
                        es = esp.tile([128, 2 * QW], bf16, tag="es")
                        nc.scalar.activation(es[:], s_ps[:], AF.Exp,
                                             scale=SCALE)
                        if kp == 0:
                            nc.vector.tensor_copy(acc2[:], es[:])
                        else:
                            nc.vector.tensor_add(acc2[:], acc2[:], es[:])
                        es_prev = (es, kp)
                    epv, kpp = es_prev
                    for i, kt in ((0, 2 * kpp), (1, 2 * kpp + 1)):
                        nc.tensor.matmul(
                            oT_ps[:], vN[kt][:], epv[:, i * QW:(i + 1) * QW],
                            start=False, stop=(kt == ST - 1))
                    # row sums: ones stationary (1-column weight load),
                    # both acc halves accumulate into one [1, 512] bank
                    rs_ps = fp.tile([1, QW], f32, tag="fin")
                    nc.tensor.matmul(rs_ps[:], ones[:], acc2[:, :QW],
                                     start=True, stop=False)
                    nc.tensor.matmul(rs_ps[:], ones[:], acc2[:, QW:],
                                     start=False, stop=True)
                    rs_row = smp.tile([1, QW], f32, tag="rsrow")
                    nc.vector.tensor_copy(rs_row[:], rs_ps[:])
                    oT_sb = osp.tile([128, QW], bf16, tag="ot")
                    nc.vector.tensor_copy(oT_sb[:], oT_ps[:])
                    for st in range(4):
                        rsT_ps = fp.tile([128, 1], f32, tag="fin")
                        nc.tensor.transpose(
                            rsT_ps[:], rs_row[:, st * 128:(st + 1) * 128],
                            ident1[:])
                        rcpT = smp.tile([128, 1], f32, tag="rcp")
                        nc.vector.reciprocal(rcpT[:], rsT_ps[:])
                        ot_ps = fp.tile([128, 128], bf16, tag="fin")
                        nc.tensor.transpose(
                            ot_ps[:], oT_sb[:, st * 128:(st + 1) * 128],
                            identb[:])
                        o_sb = finp.tile([128, 128], f32, tag="osb")
                        nc.vector.tensor_scalar_mul(o_sb[:], ot_ps[:], rcpT[:])
                        r0 = (qb * 4 + st) * 128
                        nc.sync.dma_start(
                            out=out_d[r0:r0 + 128, :], in_=o_sb[:])

    nc.finalize()
    return nc


def _get_nc():
    if "nc" not in _CACHE:
        _CACHE["nc"] = _build()
    return _CACHE["nc"]


def kernel(x, enc_output, Wq, bq, Wk, bk, Wv, bv):
    import ml_dtypes
    from concourse.bass_utils import run_bass_kernel_spmd

    bfloat16 = ml_dtypes.bfloat16
    nc = _get_nc()
    x = np.asarray(x, dtype=np.float32)
    Wqb = np.ascontiguousarray(np.asarray(Wq, np.float32).astype(bfloat16))
    Wkb = np.ascontiguousarray(np.asarray(Wk, np.float32).astype(bfloat16))
    Wvb = np.ascontiguousarray(np.asarray(Wv, np.float32).astype(bfloat16))
    in_maps = []
    for b in range(NCORES):
        in_maps.append({
            "xT": np.ascontiguousarray(x[b].T.astype(bfloat16)),
            "Wq": Wqb,
            "bq": np.asarray(bq, np.float32),
            "Wk": Wkb,
            "bk": np.asarray(bk, np.float32),
            "Wv": Wvb,
            "bv": np.asarray(bv, np.float32),
        })
    res = run_bass_kernel_spmd(nc, in_maps, list(range(NCORES)))
    out = np.stack([res.results[b]["out"] for b in range(NCORES)], axis=0)
    return out.astype(np.float32)


# revision 5
# speedup vs baseline: 1.0867x; 1.0867x over previous
"""CrossAttentionHead TRN2 kernel.

Full inputs -> full output. Shards batch (B=8) across 8 NeuronCores,
one batch element per core (pure data parallel, no collectives).

Pipeline-dense single-pass structure (all matmul operands bf16):
  lead-in: xT streamed chunk-by-chunk (DMAs issued from several engine
    queues in parallel); k (all 4 blocks), q0, q1, v0, v1 projections
    accumulate chunk-outer in 8 PSUM banks; biases on DVE.
  main loop (4 q-blocks x 8 kt-pairs): scores pair -> exp (ScalarE,
    the pacing engine) -> PV pair lagged one iteration so the exp wait
    never blocks the next scores in PE program order. Remaining work
    (v2/v3 projections, all 16 vN transposes, q2/q3 projections, the
    previous q-block's rowsum/normalize chain) is woven in as paced PE
    filler, keeping the PE busy so the HAM clock gate stays at 8/8.
  rowsum via ones-matmul on bf16 accumulators; output normalized,
  transposed back on the PE and DMA'd out one batched [512,128] store
  per q-block.
"""

import sys

if '/opt/trn_rl_repo' not in sys.path:
    sys.path.insert(0, '/opt/trn_rl_repo')

import numpy as np

B, S, E, H = 8, 2048, 768, 128
NCORES = 8
ST = S // 128          # 16 sequence tiles
EC = E // 128          # 6 embed chunks
QB = 4                 # sq blocks
QW = S // QB           # 512 sq block width
SCALE = float(1.0 / np.sqrt(np.float32(E)))

_CACHE = {}


def _build():
    import concourse.bacc as bacc
    import concourse.mybir as mybir
    import concourse.tile as tile
    from concourse.masks import make_identity

    dt = mybir.dt
    f32 = dt.float32
    bf16 = dt.bfloat16
    AF = mybir.ActivationFunctionType

    nc = bacc.Bacc(None, target_bir_lowering=False)
    xT_d = nc.dram_tensor("xT", [E, S], bf16, kind="ExternalInput")
    w_d = {}
    b_d = {}
    for nm in ("q", "k", "v"):
        w_d[nm] = nc.dram_tensor(f"W{nm}", [E, H], bf16, kind="ExternalInput")
        b_d[nm] = nc.dram_tensor(f"b{nm}", [H], f32, kind="ExternalInput")
    out_d = nc.dram_tensor("out", [S, H], f32, kind="ExternalOutput")

    with tile.TileContext(nc) as tc:
        with tc.tile_pool(name="const", bufs=1) as constp, \
             tc.tile_pool(name="big", bufs=1) as bigp:
            identb = constp.tile([128, 128], bf16)
            make_identity(nc, identb[:])
            ident1 = constp.tile([1, 1], f32)
            nc.vector.memset(ident1[:], 1.0)
            ones = constp.tile([128, 1], bf16)
            nc.vector.memset(ones[:], 1.0)
            junk = constp.tile([128, 512], bf16, name="warm_junk")
            nc.vector.memset(junk[:], 0.0)

            # --- DMA issue, spread across engine queues ---------------
            # xT: 6 single-chunk DMAs on the sync queue (sequential on
            # one DMA queue -> chunk c lands ~1.5us after chunk c-1,
            # feeding the chunk-outer projections).
            xT = []
            for c in range(EC):
                t = bigp.tile([128, S], bf16, name=f"xT{c}")
                nc.sync.dma_start(
                    out=t[:], in_=xT_d[c * 128:(c + 1) * 128, :])
                xT.append(t)
            # weights from the gpsimd queue (idle otherwise)
            w_mm = {}
            for nm in ("q", "k", "v"):
                w_mm[nm] = constp.tile([128, EC, H], bf16, name=f"w_{nm}")
                nc.gpsimd.dma_start(
                    out=w_mm[nm][:],
                    in_=w_d[nm].rearrange("(c p) d -> p c d", p=128))
            # biases from the scalar queue (idle during lead-in)
            b_sb = {}
            for nm in ("q", "k", "v"):
                b_sb[nm] = constp.tile([128, 1], f32, name=f"b_{nm}")
                nc.scalar.dma_start(out=b_sb[nm][:], in_=b_d[nm][:, None])

            # HAM warm-up while the first chunks stream in
            with tc.tile_pool(name="warm_ps", bufs=1, space="PSUM") as wmp:
                wps = wmp.tile([128, 512], f32, tag="warm")
                for _ in range(8):
                    nc.tensor.matmul(wps[:], identb[:], junk[:],
                                     start=True, stop=True)
                wsb = constp.tile([128, 512], f32, name="warm_sink")
                nc.vector.tensor_copy(wsb[:], wps[:])

            # --- lead-in projections: k0-3, q0, q1, v0, v1 ------------
            qT = [bigp.tile([128, QW], bf16, name=f"qT{n}") for n in range(4)]
            kT = [bigp.tile([128, QW], bf16, name=f"kT{n}") for n in range(4)]
            vT = [bigp.tile([128, QW], bf16, name=f"vT{n}") for n in range(4)]
            vN = [bigp.tile([128, H], bf16, name=f"vN{t}") for t in range(ST)]

            LEAD = ([("k", n) for n in range(4)]
                    + [("q", 0), ("q", 1), ("v", 0), ("v", 1)])
            with tc.tile_pool(name="lead_ps", bufs=1, space="PSUM") as leadp:
                ps = {key: leadp.tile([128, QW], f32, name=f"ps_{key[0]}{key[1]}",
                                      tag=f"t{key[0]}{key[1]}")
                      for key in LEAD}
                for c in range(EC):
                    for nm, n in LEAD:
                        nc.tensor.matmul(
                            ps[(nm, n)][:], w_mm[nm][:, c, :],
                            xT[c][:, n * 512:(n + 1) * 512],
                            start=(c == 0), stop=(c == EC - 1))
                # bias adds on DVE; k0/q0 first so scores can start
                dstmap = {"q": qT, "k": kT, "v": vT}
                for nm, n in (("k", 0), ("q", 0), ("k", 1), ("k", 2),
                              ("k", 3), ("q", 1), ("v", 0), ("v", 1)):
                    nc.vector.tensor_scalar_add(
                        dstmap[nm][n][:], ps[(nm, n)][:], b_sb[nm][:])

            # ----------------- main attention loop --------------------
            with tc.tile_pool(name="s_ps", bufs=2, space="PSUM") as sp, \
                 tc.tile_pool(name="o_ps", bufs=2, space="PSUM") as op, \
                 tc.tile_pool(name="pj_ps", bufs=1, space="PSUM") as pjp, \
                 tc.tile_pool(name="vf_ps", bufs=1, space="PSUM") as vfp, \
                 tc.tile_pool(name="es_sb", bufs=4) as esp, \
                 tc.tile_pool(name="acc_sb", bufs=3) as accp, \
                 tc.tile_pool(name="o_sb", bufs=2) as osp, \
                 tc.tile_pool(name="small", bufs=4) as smp, \
                 tc.tile_pool(name="fin", bufs=2) as finp:

                # ---- filler work generators --------------------------
                pj_state = {}

                def proj_mm(nm, n, c):
                    # chunk c of projection (nm, n) into the shared pj bank
                    if c == 0:
                        pj_state[(nm, n)] = pjp.tile(
                            [128, QW], f32, name=f"pj_{nm}{n}", tag="pj")
                    nc.tensor.matmul(
                        pj_state[(nm, n)][:], w_mm[nm][:, c, :],
                        xT[c][:, n * 512:(n + 1) * 512],
                        start=(c == 0), stop=(c == EC - 1))
                    if c == EC - 1:
                        dst = {"q": qT, "v": vT}[nm]
                        nc.vector.tensor_scalar_add(
                            dst[n][:], pj_state[(nm, n)][:], b_sb[nm][:])

                def vn_transpose(t):
                    pt = vfp.tile([128, 128], bf16, tag="vf")
                    nc.tensor.transpose(
                        pt[:], vT[t // 4][:, (t % 4) * 128:(t % 4 + 1) * 128],
                        identb[:])
                    if t % 2 == 0:
                        nc.vector.tensor_copy(vN[t][:], pt[:])
                    else:
                        nc.scalar.copy(vN[t][:], pt[:])

                def filler_gen():
                    # qb0 fillers: v2/v3 projections + all 16 vN transposes
                    # paced so vN[t] exists before PV consumes it.
                    for i in range(6):
                        yield [("T", 2 * i), ("T", 2 * i + 1),
                               ("P", ("v", 2 + i // 3, i % 3 * 2)),
                               ("P", ("v", 2 + i // 3, i % 3 * 2 + 1))]
                    yield [("T", 12), ("T", 13)]
                    yield [("T", 14), ("T", 15)]
                    # qb1 fillers: q2 projection
                    for i in range(3):
                        yield [("P", ("q", 2, 2 * i)), ("P", ("q", 2, 2 * i + 1))]
                    for _ in range(5):
                        yield []
                    # qb2 fillers: q3 projection
                    for i in range(3):
                        yield [("P", ("q", 3, 2 * i)), ("P", ("q", 3, 2 * i + 1))]
                    while True:
                        yield []

                fill = filler_gen()

                def run_filler(items):
                    for kind, a in items:
                        if kind == "T":
                            vn_transpose(a)
                        else:
                            proj_mm(*a)

                # ---- finalize chain for one q-block, as step closures -
                def fin_steps(qb, acc2, oT_ps):
                    # step 0: rowsums + copies
                    rs_ps = vfp.tile([1, QW], f32, tag="vf")
                    nc.tensor.matmul(rs_ps[:], ones[:], acc2[:, :QW],
                                     start=True, stop=False)
                    nc.tensor.matmul(rs_ps[:], ones[:], acc2[:, QW:],
                                     start=False, stop=True)
                    rs_row = smp.tile([1, QW], f32, tag="rsrow")
                    nc.vector.tensor_copy(rs_row[:], rs_ps[:])
                    oT_sb = osp.tile([128, QW], bf16, tag="ot")
                    nc.vector.tensor_copy(oT_sb[:], oT_ps[:])
                    o_sb4 = finp.tile([128, 4, 128], f32, tag="osb")
                    yield
                    for st in range(4):
                        rsT_ps = vfp.tile([128, 1], f32, tag="vf")
                        nc.tensor.transpose(
                            rsT_ps[:], rs_row[:, st * 128:(st + 1) * 128],
                            ident1[:])
                        rcpT = smp.tile([128, 1], f32, tag="rcp")
                        nc.vector.reciprocal(rcpT[:], rsT_ps[:])
                        ot_ps = vfp.tile([128, 128], bf16, tag="vf")
                        nc.tensor.transpose(
                            ot_ps[:], oT_sb[:, st * 128:(st + 1) * 128],
                            identb[:])
                        nc.vector.tensor_scalar_mul(
                            o_sb4[:, st, :], ot_ps[:], rcpT[:])
                        yield
                    nc.sync.dma_start(
                        out=out_d.rearrange("(b t p) h -> b p t h",
                                            t=4, p=128)[qb],
                        in_=o_sb4[:])
                    while True:
                        yield

                pending_fin = None
                for qb in range(QB):
                    oT_ps = op.tile([128, QW], f32, tag="opv")
                    acc2 = accp.tile([128, 2 * QW], bf16, tag="acc")
                    es_prev = None
                    for kp in range(ST // 2):
                        run_filler(next(fill))
                        if pending_fin is not None:
                            next(pending_fin)
                        kt0, kt1 = 2 * kp, 2 * kp + 1
                        s_ps = sp.tile([128, 2 * QW], f32, tag="s")
                        for i, kt in ((0, kt0), (1, kt1)):
                            nc.tensor.matmul(
                                s_ps[:, i * QW:(i + 1) * QW],
                                kT[kt // 4][:, (kt % 4) * 128:(kt % 4 + 1) * 128],
                                qT[qb][:], start=True, stop=True)
                        if es_prev is not None:
                            epv, kpp = es_prev
                            for i, kt in ((0, 2 * kpp), (1, 2 * kpp + 1)):
                                nc.tensor.matmul(
                                    oT_ps[:], vN[kt][:],
                                    epv[:, i * QW:(i + 1) * QW],
                                    start=(kt == 0), stop=False)
                        es = esp.tile([128, 2 * QW], bf16, tag="es")
                        nc.scalar.activation(es[:], s_ps[:], AF.Exp,
                                             scale=SCALE)
                        if kp == 0:
                            nc.vector.tensor_copy(acc2[:], es[:])
                        else:
                            nc.vector.tensor_add(acc2[:], acc2[:], es[:])
                        es_prev = (es, kp)
                    epv, kpp = es_prev
                    for i, kt in ((0, 2 * kpp), (1, 2 * kpp + 1)):
                        nc.tensor.matmul(
                            oT_ps[:], vN[kt][:], epv[:, i * QW:(i + 1) * QW],
                            start=False, stop=(kt == ST - 1))
                    pending_fin = fin_steps(qb, acc2, oT_ps)
                # drain the last q-block's finalize chain
                for _ in range(6):
                    next(pending_fin)

    nc.finalize()
    return nc


def _get_nc():
    if "nc" not in _CACHE:
        _CACHE["nc"] = _build()
    return _CACHE["nc"]


def kernel(x, enc_output, Wq, bq, Wk, bk, Wv, bv):
    import ml_dtypes
    from concourse.bass_utils import run_bass_kernel_spmd

    bfloat16 = ml_dtypes.bfloat16
    nc = _get_nc()
    x = np.asarray(x, dtype=np.float32)
    Wqb = np.ascontiguousarray(np.asarray(Wq, np.float32).astype(bfloat16))
    Wkb = np.ascontiguousarray(np.asarray(Wk, np.float32).astype(bfloat16))
    Wvb = np.ascontiguousarray(np.asarray(Wv, np.float32).astype(bfloat16))
    in_maps = []
    for b in range(NCORES):
        in_maps.append({
            "xT": np.ascontiguousarray(x[b].T.astype(bfloat16)),
            "Wq": Wqb,
            "bq": np.asarray(bq, np.float32),
            "Wk": Wkb,
            "bk": np.asarray(bk, np.float32),
            "Wv": Wvb,
            "bv": np.asarray(bv, np.float32),
        })
    res = run_bass_kernel_spmd(nc, in_maps, list(range(NCORES)))
    out = np.stack([res.results[b]["out"] for b in range(NCORES)], axis=0)
    return out.astype(np.float32)


# revision 7
# speedup vs baseline: 1.1211x; 1.0317x over previous
"""CrossAttentionHead TRN2 kernel.

Full inputs -> full output. Shards batch (B=8) across 8 NeuronCores,
one batch element per core (pure data parallel, no collectives).

Pipeline-dense single-pass structure (all matmul operands bf16):
  lead-in: xT streamed chunk-by-chunk (DMAs issued from several engine
    queues in parallel); k (all 4 blocks), q0, q1, v0, v1 projections
    accumulate chunk-outer in 8 PSUM banks; biases on DVE.
  main loop (4 q-blocks x 8 kt-pairs): scores pair -> exp (ScalarE,
    the pacing engine) -> PV pair lagged one iteration so the exp wait
    never blocks the next scores in PE program order. Remaining work
    (v2/v3 projections, all 16 vN transposes, q2/q3 projections, the
    previous q-block's rowsum/normalize chain) is woven in as paced PE
    filler, keeping the PE busy so the HAM clock gate stays at 8/8.
  rowsum via ones-matmul on bf16 accumulators; output normalized,
  transposed back on the PE and DMA'd out one batched [512,128] store
  per q-block.
"""

import sys

if '/opt/trn_rl_repo' not in sys.path:
    sys.path.insert(0, '/opt/trn_rl_repo')

import numpy as np

B, S, E, H = 8, 2048, 768, 128
NCORES = 8
ST = S // 128          # 16 sequence tiles
EC = E // 128          # 6 embed chunks
QB = 4                 # sq blocks
QW = S // QB           # 512 sq block width
SCALE = float(1.0 / np.sqrt(np.float32(E)))

_CACHE = {}


def _build():
    import concourse.bacc as bacc
    import concourse.mybir as mybir
    import concourse.tile as tile
    from concourse.masks import make_identity

    dt = mybir.dt
    f32 = dt.float32
    bf16 = dt.bfloat16
    AF = mybir.ActivationFunctionType

    nc = bacc.Bacc(None, target_bir_lowering=False)
    xT_d = nc.dram_tensor("xT", [E, S], bf16, kind="ExternalInput")
    w_d = {}
    b_d = {}
    for nm in ("q", "k", "v"):
        w_d[nm] = nc.dram_tensor(f"W{nm}", [E, H], bf16, kind="ExternalInput")
        b_d[nm] = nc.dram_tensor(f"b{nm}", [H], f32, kind="ExternalInput")
    out_d = nc.dram_tensor("out", [S, H], f32, kind="ExternalOutput")

    with tile.TileContext(nc) as tc:
        with tc.tile_pool(name="const", bufs=1) as constp, \
             tc.tile_pool(name="big", bufs=1) as bigp:
            junk = constp.tile([128, 512], bf16, name="warm_junk")
            nc.vector.memset(junk[:], 0.0)
            ident1 = constp.tile([1, 1], f32)
            nc.vector.memset(ident1[:], 1.0)
            ones = constp.tile([128, 1], bf16)
            nc.vector.memset(ones[:], 1.0)

            # --- DMA issue, spread across engine queues ---------------
            # xT chunks alternate between the sync and scalar DMA queues
            # so issue+transfer pipelines two streams deep.
            xT = []
            for c in range(EC):
                t = bigp.tile([128, S], bf16, name=f"xT{c}")
                eng = nc.sync if c % 2 == 0 else nc.scalar
                eng.dma_start(out=t[:], in_=xT_d[c * 128:(c + 1) * 128, :])
                xT.append(t)
            # weights from the gpsimd queue, BEFORE the identity work so
            # their issue isn't delayed behind it
            w_mm = {}
            for nm in ("q", "k", "v"):
                w_mm[nm] = constp.tile([128, EC, H], bf16, name=f"w_{nm}")
                nc.gpsimd.dma_start(
                    out=w_mm[nm][:],
                    in_=w_d[nm].rearrange("(c p) d -> p c d", p=128))
            b_sb = {}
            for nm in ("q", "k", "v"):
                b_sb[nm] = constp.tile([128, 1], f32, name=f"b_{nm}")
                nc.gpsimd.dma_start(out=b_sb[nm][:], in_=b_d[nm][:, None])

            identb = constp.tile([128, 128], bf16)
            make_identity(nc, identb[:])

            # HAM warm-up while the first chunks stream in: junk-on-junk
            # matmuls depend only on the vector memset, so the PE starts
            # flipping the clock gate right after the preamble.
            with tc.tile_pool(name="warm_ps", bufs=1, space="PSUM") as wmp:
                wps = wmp.tile([128, 512], f32, tag="warm")
                for _ in range(12):
                    nc.tensor.matmul(wps[:], junk[:, :128], junk[:],
                                     start=True, stop=True)
                wsb = constp.tile([128, 512], f32, name="warm_sink")
                nc.vector.tensor_copy(wsb[:], wps[:])

            # --- lead-in projections: k0-3, q0, q1, v0, v1 ------------
            qT = [bigp.tile([128, QW], bf16, name=f"qT{n}") for n in range(4)]
            kT = [bigp.tile([128, QW], bf16, name=f"kT{n}") for n in range(4)]
            vT = [bigp.tile([128, QW], bf16, name=f"vT{n}") for n in range(4)]
            vN = [bigp.tile([128, H], bf16, name=f"vN{t}") for t in range(ST)]

            LEAD = ([("k", n) for n in range(4)]
                    + [("q", 0), ("q", 1), ("v", 0), ("v", 1)])
            with tc.tile_pool(name="lead_ps", bufs=1, space="PSUM") as leadp:
                ps = {key: leadp.tile([128, QW], f32, name=f"ps_{key[0]}{key[1]}",
                                      tag=f"t{key[0]}{key[1]}")
                      for key in LEAD}
                for c in range(EC):
                    for nm, n in LEAD:
                        nc.tensor.matmul(
                            ps[(nm, n)][:], w_mm[nm][:, c, :],
                            xT[c][:, n * 512:(n + 1) * 512],
                            start=(c == 0), stop=(c == EC - 1))
                # bias adds on DVE; k0/q0 first so scores can start
                dstmap = {"q": qT, "k": kT, "v": vT}
                for nm, n in (("k", 0), ("q", 0), ("k", 1), ("k", 2),
                              ("k", 3), ("q", 1), ("v", 0), ("v", 1)):
                    nc.vector.tensor_scalar_add(
                        dstmap[nm][n][:], ps[(nm, n)][:], b_sb[nm][:])

            # ----------------- main attention loop --------------------
            with tc.tile_pool(name="s_ps", bufs=2, space="PSUM") as sp, \
                 tc.tile_pool(name="o_ps", bufs=2, space="PSUM") as op, \
                 tc.tile_pool(name="pj_ps", bufs=1, space="PSUM") as pjp, \
                 tc.tile_pool(name="vf_ps", bufs=1, space="PSUM") as vfp, \
                 tc.tile_pool(name="es_sb", bufs=4) as esp, \
                 tc.tile_pool(name="acc_sb", bufs=3) as accp, \
                 tc.tile_pool(name="o_sb", bufs=2) as osp, \
                 tc.tile_pool(name="small", bufs=4) as smp, \
                 tc.tile_pool(name="fin", bufs=2) as finp:

                # ---- filler work generators --------------------------
                pj_state = {}

                def proj_mm(nm, n, c):
                    # chunk c of projection (nm, n) into the shared pj bank
                    if c == 0:
                        pj_state[(nm, n)] = pjp.tile(
                            [128, QW], f32, name=f"pj_{nm}{n}", tag="pj")
                    nc.tensor.matmul(
                        pj_state[(nm, n)][:], w_mm[nm][:, c, :],
                        xT[c][:, n * 512:(n + 1) * 512],
                        start=(c == 0), stop=(c == EC - 1))
                    if c == EC - 1:
                        dst = {"q": qT, "v": vT}[nm]
                        nc.vector.tensor_scalar_add(
                            dst[n][:], pj_state[(nm, n)][:], b_sb[nm][:])

                def vn_transpose(t, scalar_copy=False):
                    pt = vfp.tile([128, 128], bf16, tag="vf")
                    nc.tensor.transpose(
                        pt[:], vT[t // 4][:, (t % 4) * 128:(t % 4 + 1) * 128],
                        identb[:])
                    if scalar_copy:
                        nc.scalar.copy(vN[t][:], pt[:])
                    else:
                        nc.vector.tensor_copy(vN[t][:], pt[:])

                # v0/v1 transposes as a pre-loop burst (ScalarE still
                # idle here, so half the copies go to it for free)
                for t in range(8):
                    vn_transpose(t, scalar_copy=bool(t % 2))

                def filler_gen():
                    # qb0 fillers: v2/v3 projections + their vN transposes
                    # paced so vN[t] exists before PV consumes it.
                    yield [("P", ("v", 2, 0)), ("P", ("v", 2, 1))]
                    yield [("P", ("v", 2, 2)), ("P", ("v", 2, 3))]
                    yield [("P", ("v", 2, 4)), ("P", ("v", 2, 5))]
                    yield [("P", ("v", 3, 0)), ("P", ("v", 3, 1)), ("T", 8)]
                    yield [("P", ("v", 3, 2)), ("P", ("v", 3, 3)),
                           ("T", 9), ("T", 10)]
                    yield [("P", ("v", 3, 4)), ("P", ("v", 3, 5)), ("T", 11)]
                    yield [("T", 12), ("T", 13)]
                    yield [("T", 14), ("T", 15)]
                    # qb1 fillers: q2 projection
                    for i in range(3):
                        yield [("P", ("q", 2, 2 * i)), ("P", ("q", 2, 2 * i + 1))]
                    for _ in range(5):
                        yield []
                    # qb2 fillers: q3 projection
                    for i in range(3):
                        yield [("P", ("q", 3, 2 * i)), ("P", ("q", 3, 2 * i + 1))]
                    while True:
                        yield []

                fill = filler_gen()

                def run_filler(items):
                    for kind, a in items:
                        if kind == "T":
                            vn_transpose(a)
                        else:
                            proj_mm(*a)

                # ---- finalize chain for one q-block, as step closures -
                def fin_steps(qb, acc2, oT_ps):
                    # step 0: rowsums + copies
                    rs_ps = vfp.tile([1, QW], f32, tag="vf")
                    nc.tensor.matmul(rs_ps[:], ones[:], acc2[:, :QW],
                                     start=True, stop=False)
                    nc.tensor.matmul(rs_ps[:], ones[:], acc2[:, QW:],
                                     start=False, stop=True)
                    rs_row = smp.tile([1, QW], f32, tag="rsrow")
                    nc.vector.tensor_copy(rs_row[:], rs_ps[:])
                    oT_sb = osp.tile([128, QW], bf16, tag="ot")
                    nc.vector.tensor_copy(oT_sb[:], oT_ps[:])
                    o_sb4 = finp.tile([128, 4, 128], f32, tag="osb")
                    yield
                    for st in range(4):
                        rsT_ps = vfp.tile([128, 1], f32, tag="vf")
                        nc.tensor.transpose(
                            rsT_ps[:], rs_row[:, st * 128:(st + 1) * 128],
                            ident1[:])
                        rcpT = smp.tile([128, 1], f32, tag="rcp")
                        nc.vector.reciprocal(rcpT[:], rsT_ps[:])
                        ot_ps = vfp.tile([128, 128], bf16, tag="vf")
                        nc.tensor.transpose(
                            ot_ps[:], oT_sb[:, st * 128:(st + 1) * 128],
                            identb[:])
                        nc.vector.tensor_scalar_mul(
                            o_sb4[:, st, :], ot_ps[:], rcpT[:])
                        yield
                    nc.sync.dma_start(
                        out=out_d.rearrange("(b t p) h -> b p t h",
                                            t=4, p=128)[qb],
                        in_=o_sb4[:])
                    while True:
                        yield

                pending_fin = None
                for qb in range(QB):
                    oT_ps = op.tile([128, QW], f32, tag="opv")
                    acc2 = accp.tile([128, 2 * QW], bf16, tag="acc")
                    es_prev = None
                    for kp in range(ST // 2):
                        run_filler(next(fill))
                        if pending_fin is not None:
                            next(pending_fin)
                        kt0, kt1 = 2 * kp, 2 * kp + 1
                        s_ps = sp.tile([128, 2 * QW], f32, tag="s")
                        for i, kt in ((0, kt0), (1, kt1)):
                            nc.tensor.matmul(
                                s_ps[:, i * QW:(i + 1) * QW],
                                kT[kt // 4][:, (kt % 4) * 128:(kt % 4 + 1) * 128],
                                qT[qb][:], start=True, stop=True)
                        if es_prev is not None:
                            epv, kpp = es_prev
                            for i, kt in ((0, 2 * kpp), (1, 2 * kpp + 1)):
                                nc.tensor.matmul(
                                    oT_ps[:], vN[kt][:],
                                    epv[:, i * QW:(i + 1) * QW],
                                    start=(kt == 0), stop=False)
                        es = esp.tile([128, 2 * QW], bf16, tag="es")
                        nc.scalar.activation(es[:], s_ps[:], AF.Exp,
                                             scale=SCALE)
                        if kp == 0:
                            nc.vector.tensor_copy(acc2[:], es[:])
                        else:
                            nc.vector.tensor_add(acc2[:], acc2[:], es[:])
                        es_prev = (es, kp)
                    epv, kpp = es_prev
                    for i, kt in ((0, 2 * kpp), (1, 2 * kpp + 1)):
                        nc.tensor.matmul(
                            oT_ps[:], vN[kt][:], epv[:, i * QW:(i + 1) * QW],
                            start=False, stop=(kt == ST - 1))
                    pending_fin = fin_steps(qb, acc2, oT_ps)
                # drain the last q-block's finalize chain
                for _ in range(6):
                    next(pending_fin)

    nc.finalize()
    return nc


def _get_nc():
    if "nc" not in _CACHE:
        _CACHE["nc"] = _build()
    return _CACHE["nc"]


def kernel(x, enc_output, Wq, bq, Wk, bk, Wv, bv):
    import ml_dtypes
    from concourse.bass_utils import run_bass_kernel_spmd

    bfloat16 = ml_dtypes.bfloat16
    nc = _get_nc()
    x = np.asarray(x, dtype=np.float32)
    Wqb = np.ascontiguousarray(np.asarray(Wq, np.float32).astype(bfloat16))
    Wkb = np.ascontiguousarray(np.asarray(Wk, np.float32).astype(bfloat16))
    Wvb = np.ascontiguousarray(np.asarray(Wv, np.float32).astype(bfloat16))
    in_maps = []
    for b in range(NCORES):
        in_maps.append({
            "xT": np.ascontiguousarray(x[b].T.astype(bfloat16)),
            "Wq": Wqb,
            "bq": np.asarray(bq, np.float32),
            "Wk": Wkb,
            "bk": np.asarray(bk, np.float32),
            "Wv": Wvb,
            "bv": np.asarray(bv, np.float32),
        })
    res = run_bass_kernel_spmd(nc, in_maps, list(range(NCORES)))
    out = np.stack([res.results[b]["out"] for b in range(NCORES)], axis=0)
    return out.astype(np.float32)


# revision 9
# speedup vs baseline: 1.1976x; 1.0682x over previous
"""CrossAttentionHead TRN2 kernel.

Full inputs -> full output. Shards batch (B=8) across 8 NeuronCores,
one batch element per core (pure data parallel, no collectives).

Pipeline-dense single-pass structure (all matmul operands bf16):
  lead-in: xT streamed chunk-by-chunk (DMAs issued from several engine
    queues in parallel); k (all 4 blocks), q0, q1, v0, v1 projections
    accumulate chunk-outer in 8 PSUM banks; biases on DVE.
  main loop (4 q-blocks x 8 kt-pairs): scores pair -> exp (ScalarE,
    the pacing engine) -> PV pair lagged one iteration so the exp wait
    never blocks the next scores in PE program order. Remaining work
    (v2/v3 projections, all 16 vN transposes, q2/q3 projections, the
    previous q-block's rowsum/normalize chain) is woven in as paced PE
    filler, keeping the PE busy so the HAM clock gate stays at 8/8.
  rowsum via ones-matmul on bf16 accumulators; output normalized,
  transposed back on the PE and DMA'd out one batched [512,128] store
  per q-block.
"""

import sys

if '/opt/trn_rl_repo' not in sys.path:
    sys.path.insert(0, '/opt/trn_rl_repo')

import numpy as np

B, S, E, H = 8, 2048, 768, 128
NCORES = 8
ST = S // 128          # 16 sequence tiles
EC = E // 128          # 6 embed chunks
QB = 4                 # sq blocks
QW = S // QB           # 512 sq block width
SCALE = float(1.0 / np.sqrt(np.float32(E)))

_CACHE = {}


def _build():
    import concourse.bacc as bacc
    import concourse.mybir as mybir
    import concourse.tile as tile
    from concourse.masks import make_identity

    dt = mybir.dt
    f32 = dt.float32
    bf16 = dt.bfloat16
    AF = mybir.ActivationFunctionType

    nc = bacc.Bacc(None, target_bir_lowering=False)
    xT_d = nc.dram_tensor("xT", [E, S], bf16, kind="ExternalInput")
    w_d = {}
    b_d = {}
    for nm in ("q", "k", "v"):
        w_d[nm] = nc.dram_tensor(f"W{nm}", [E, H], bf16, kind="ExternalInput")
        b_d[nm] = nc.dram_tensor(f"b{nm}", [H], f32, kind="ExternalInput")
    out_d = nc.dram_tensor("out", [S, H], f32, kind="ExternalOutput")

    with tile.TileContext(nc) as tc:
        with tc.tile_pool(name="const", bufs=1) as constp, \
             tc.tile_pool(name="big", bufs=1) as bigp:
            junk = constp.tile([128, 512], bf16, name="warm_junk")
            nc.vector.memset(junk[:], 0.0)
            ident1 = constp.tile([1, 1], f32)
            nc.vector.memset(ident1[:], 1.0)
            ones = constp.tile([128, 1], bf16)
            nc.vector.memset(ones[:], 1.0)

            # --- DMA issue ordering matters: all DMAs share one HW
            # queue FIFO, so weights+biases go FIRST (small, needed by
            # the first projection matmul), then the xT chunks in
            # consumption order.
            w_mm = {}
            for nm in ("q", "k", "v"):
                w_mm[nm] = constp.tile([128, EC, H], bf16, name=f"w_{nm}")
                nc.sync.dma_start(
                    out=w_mm[nm][:],
                    in_=w_d[nm].rearrange("(c p) d -> p c d", p=128))
            b_sb = {}
            for nm in ("q", "k", "v"):
                b_sb[nm] = constp.tile([128, 1], f32, name=f"b_{nm}")
                nc.sync.dma_start(out=b_sb[nm][:], in_=b_d[nm][:, None])
            xT = []
            for c in range(EC):
                t = bigp.tile([128, S], bf16, name=f"xT{c}")
                nc.sync.dma_start(out=t[:], in_=xT_d[c * 128:(c + 1) * 128, :])
                xT.append(t)

            identb = constp.tile([128, 128], bf16)
            make_identity(nc, identb[:])

            # HAM warm-up while the first chunks stream in: junk-on-junk
            # matmuls depend only on the vector memset, so the PE starts
            # flipping the clock gate right after the preamble.
            with tc.tile_pool(name="warm_ps", bufs=1, space="PSUM") as wmp:
                wps = wmp.tile([128, 512], f32, tag="warm")
                for _ in range(12):
                    nc.tensor.matmul(wps[:], junk[:, :128], junk[:],
                                     start=True, stop=True)
                wsb = constp.tile([128, 512], f32, name="warm_sink")
                nc.vector.tensor_copy(wsb[:], wps[:])

            # --- lead-in projections: k0-3, q0, q1, v0, v1 ------------
            qT = [bigp.tile([128, QW], bf16, name=f"qT{n}") for n in range(4)]
            kT = [bigp.tile([128, QW], bf16, name=f"kT{n}") for n in range(4)]
            vT = [bigp.tile([128, QW], bf16, name=f"vT{n}") for n in range(4)]
            vN = [bigp.tile([128, H], bf16, name=f"vN{t}") for t in range(ST)]

            LEAD = ([("k", n) for n in range(4)]
                    + [("q", 0), ("q", 1), ("v", 0), ("v", 1)])
            with tc.tile_pool(name="lead_ps", bufs=1, space="PSUM") as leadp:
                ps = {key: leadp.tile([128, QW], f32, name=f"ps_{key[0]}{key[1]}",
                                      tag=f"t{key[0]}{key[1]}")
                      for key in LEAD}
                for c in range(EC):
                    for nm, n in LEAD:
                        nc.tensor.matmul(
                            ps[(nm, n)][:], w_mm[nm][:, c, :],
                            xT[c][:, n * 512:(n + 1) * 512],
                            start=(c == 0), stop=(c == EC - 1))
                # bias adds split across ScalarE (k, idle pre-loop) and
                # DVE (q/v) so they run concurrently; k0/q0 first so the
                # first scores pair can start immediately.
                nc.scalar.activation(kT[0][:], ps[("k", 0)][:], AF.Identity,
                                     bias=b_sb["k"][:], scale=1.0)
                nc.vector.tensor_scalar_add(
                    qT[0][:], ps[("q", 0)][:], b_sb["q"][:])
                for n in range(1, 4):
                    nc.scalar.activation(kT[n][:], ps[("k", n)][:],
                                         AF.Identity,
                                         bias=b_sb["k"][:], scale=1.0)
                for nm, n in (("q", 1), ("v", 0), ("v", 1)):
                    nc.vector.tensor_scalar_add(
                        {"q": qT, "v": vT}[nm][n][:], ps[(nm, n)][:],
                        b_sb[nm][:])

            # ----------------- main attention loop --------------------
            with tc.tile_pool(name="s_ps", bufs=2, space="PSUM") as sp, \
                 tc.tile_pool(name="o_ps", bufs=2, space="PSUM") as op, \
                 tc.tile_pool(name="pj_ps", bufs=1, space="PSUM") as pjp, \
                 tc.tile_pool(name="vf_ps", bufs=1, space="PSUM") as vfp, \
                 tc.tile_pool(name="es_sb", bufs=4) as esp, \
                 tc.tile_pool(name="acc_sb", bufs=3) as accp, \
                 tc.tile_pool(name="o_sb", bufs=2) as osp, \
                 tc.tile_pool(name="small", bufs=4) as smp, \
                 tc.tile_pool(name="fin", bufs=2) as finp:

                # ---- filler work generators --------------------------
                pj_state = {}

                def proj_mm(nm, n, c):
                    # chunk c of projection (nm, n) into the shared pj bank
                    if c == 0:
                        pj_state[(nm, n)] = pjp.tile(
                            [128, QW], f32, name=f"pj_{nm}{n}", tag="pj")
                    nc.tensor.matmul(
                        pj_state[(nm, n)][:], w_mm[nm][:, c, :],
                        xT[c][:, n * 512:(n + 1) * 512],
                        start=(c == 0), stop=(c == EC - 1))
                    if c == EC - 1:
                        dst = {"q": qT, "v": vT}[nm]
                        nc.vector.tensor_scalar_add(
                            dst[n][:], pj_state[(nm, n)][:], b_sb[nm][:])

                def vn_transpose(t, scalar_copy=False):
                    pt = vfp.tile([128, 128], bf16, tag="vf")
                    nc.tensor.transpose(
                        pt[:], vT[t // 4][:, (t % 4) * 128:(t % 4 + 1) * 128],
                        identb[:])
                    if scalar_copy:
                        nc.scalar.copy(vN[t][:], pt[:])
                    else:
                        nc.vector.tensor_copy(vN[t][:], pt[:])

                # v0/v1 transposes as a pre-loop burst (ScalarE still
                # idle here, so half the copies go to it for free)
                for t in range(8):
                    vn_transpose(t, scalar_copy=bool(t % 2))

                def filler_gen():
                    # qb0 fillers: v2/v3 projections + their vN transposes
                    # paced so vN[t] exists before PV consumes it.
                    yield [("P", ("v", 2, 0)), ("P", ("v", 2, 1))]
                    yield [("P", ("v", 2, 2)), ("P", ("v", 2, 3))]
                    yield [("P", ("v", 2, 4)), ("P", ("v", 2, 5))]
                    yield [("P", ("v", 3, 0)), ("P", ("v", 3, 1)), ("T", 8)]
                    yield [("P", ("v", 3, 2)), ("P", ("v", 3, 3)),
                           ("T", 9), ("T", 10)]
                    yield [("P", ("v", 3, 4)), ("P", ("v", 3, 5)), ("T", 11)]
                    yield [("T", 12), ("T", 13)]
                    yield [("T", 14), ("T", 15)]
                    # qb1 fillers: q2 projection
                    for i in range(3):
                        yield [("P", ("q", 2, 2 * i)), ("P", ("q", 2, 2 * i + 1))]
                    for _ in range(5):
                        yield []
                    # qb2 fillers: q3 projection
                    for i in range(3):
                        yield [("P", ("q", 3, 2 * i)), ("P", ("q", 3, 2 * i + 1))]
                    while True:
                        yield []

                fill = filler_gen()

                def run_filler(items):
                    for kind, a in items:
                        if kind == "T":
                            vn_transpose(a)
                        else:
                            proj_mm(*a)

                # ---- finalize chain for one q-block, as step closures -
                def fin_steps(qb, acc2, oT_ps):
                    # step 0: rowsums + copies
                    rs_ps = vfp.tile([1, QW], f32, tag="vf")
                    nc.tensor.matmul(rs_ps[:], ones[:], acc2[:, :QW],
                                     start=True, stop=False)
                    nc.tensor.matmul(rs_ps[:], ones[:], acc2[:, QW:],
                                     start=False, stop=True)
                    rs_row = smp.tile([1, QW], f32, tag="rsrow")
                    nc.vector.tensor_copy(rs_row[:], rs_ps[:])
                    oT_sb = osp.tile([128, QW], bf16, tag="ot")
                    nc.vector.tensor_copy(oT_sb[:], oT_ps[:])
                    o_sb4 = finp.tile([128, 4, 128], f32, tag="osb")
                    yield
                    for st in range(4):
                        rsT_ps = vfp.tile([128, 1], f32, tag="vf")
                        nc.tensor.transpose(
                            rsT_ps[:], rs_row[:, st * 128:(st + 1) * 128],
                            ident1[:])
                        rcpT = smp.tile([128, 1], f32, tag="rcp")
                        nc.vector.reciprocal(rcpT[:], rsT_ps[:])
                        ot_ps = vfp.tile([128, 128], bf16, tag="vf")
                        nc.tensor.transpose(
                            ot_ps[:], oT_sb[:, st * 128:(st + 1) * 128],
                            identb[:])
                        nc.vector.tensor_scalar_mul(
                            o_sb4[:, st, :], ot_ps[:], rcpT[:])
                        yield
                    nc.sync.dma_start(
                        out=out_d.rearrange("(b t p) h -> b p t h",
                                            t=4, p=128)[qb],
                        in_=o_sb4[:])
                    while True:
                        yield

                pending_fin = None
                for qb in range(QB):
                    oT_ps = op.tile([128, QW], f32, tag="opv")
                    acc2 = accp.tile([128, 2 * QW], bf16, tag="acc")
                    es_prev = None
                    for kp in range(ST // 2):
                        run_filler(next(fill))
                        if pending_fin is not None:
                            next(pending_fin)
                        kt0, kt1 = 2 * kp, 2 * kp + 1
                        s_ps = sp.tile([128, 2 * QW], f32, tag="s")
                        for i, kt in ((0, kt0), (1, kt1)):
                            nc.tensor.matmul(
                                s_ps[:, i * QW:(i + 1) * QW],
                                kT[kt // 4][:, (kt % 4) * 128:(kt % 4 + 1) * 128],
                                qT[qb][:], start=True, stop=True)
                        if es_prev is not None:
                            epv, kpp = es_prev
                            for i, kt in ((0, 2 * kpp), (1, 2 * kpp + 1)):
                                nc.tensor.matmul(
                                    oT_ps[:], vN[kt][:],
                                    epv[:, i * QW:(i + 1) * QW],
                                    start=(kt == 0), stop=False)
                        es = esp.tile([128, 2 * QW], bf16, tag="es")
                        nc.scalar.activation(es[:], s_ps[:], AF.Exp,
                                             scale=SCALE)
                        if kp == 0:
                            nc.vector.tensor_copy(acc2[:], es[:])
                        else:
                            nc.vector.tensor_add(acc2[:], acc2[:], es[:])
                        es_prev = (es, kp)
                    epv, kpp = es_prev
                    for i, kt in ((0, 2 * kpp), (1, 2 * kpp + 1)):
                        nc.tensor.matmul(
                            oT_ps[:], vN[kt][:], epv[:, i * QW:(i + 1) * QW],
                            start=False, stop=(kt == ST - 1))
                    pending_fin = fin_steps(qb, acc2, oT_ps)
                # drain the last q-block's finalize chain
                for _ in range(6):
                    next(pending_fin)

    nc.finalize()
    return nc


def _get_nc():
    if "nc" not in _CACHE:
        _CACHE["nc"] = _build()
    return _CACHE["nc"]


def kernel(x, enc_output, Wq, bq, Wk, bk, Wv, bv):
    import ml_dtypes
    from concourse.bass_utils import run_bass_kernel_spmd

    bfloat16 = ml_dtypes.bfloat16
    nc = _get_nc()
    x = np.asarray(x, dtype=np.float32)
    Wqb = np.ascontiguousarray(np.asarray(Wq, np.float32).astype(bfloat16))
    Wkb = np.ascontiguousarray(np.asarray(Wk, np.float32).astype(bfloat16))
    Wvb = np.ascontiguousarray(np.asarray(Wv, np.float32).astype(bfloat16))
    in_maps = []
    for b in range(NCORES):
        in_maps.append({
            "xT": np.ascontiguousarray(x[b].T.astype(bfloat16)),
            "Wq": Wqb,
            "bq": np.asarray(bq, np.float32),
            "Wk": Wkb,
            "bk": np.asarray(bk, np.float32),
            "Wv": Wvb,
            "bv": np.asarray(bv, np.float32),
        })
    res = run_bass_kernel_spmd(nc, in_maps, list(range(NCORES)))
    out = np.stack([res.results[b]["out"] for b in range(NCORES)], axis=0)
    return out.astype(np.float32)
